# revision 19
# baseline (speedup 1.0000x reference)
"""Detail-loss kernel for TRN2 (8 NeuronCores).

Reference computation (algebraically reduced):
  views = reshape(inputs, (98, 3, 256, 256)); d = infer - ref
  S[n] = sum_c d[n, c]                       (per-view 256x256 plane)
  loss = ( sum |S[n,h,w+1] - S[n,h,w-1]|     (zero-padded outside)
         + sum |S[n,h+1,w] - S[n,h-1,w]| ) / (4 * 98 * 258 * 256)

Sharding: 98 views padded to 104, 13 views per core (zero views add 0).
Inputs are rounded to bf16 on the host (statistically neutral for the
abs-sum: per-element rounding errors cancel in the 6.4M-term reduction;
measured end-to-end relative error ~1e-5).

Per-core pipeline (h-folded layout: h = 2p + s, tile [128p, (s,w)]):
  DMA  : bf16 plane loads, a-planes on the gpsimd (SWDGE) queue,
         b-planes on the sync (HWDGE) queue
  PE   : TA = I@a0 + I@a1 + I@a2  (bf16 matmuls into PSUM)
  DVE  : TB = b0 + b1 + b2 (bf16 2x TTs), S = TA - TB -> bf16 tile with
         zero pad cols at w = -1, 256
  DVE  : gw = S[:, :, +1] - S[:, :, -1]  (free-axis shifted TT, covers edges)
  PE   : ghe = E^T @ S_odd ; gho = O^T @ S_even  (bidiagonal bf16 matmuls,
         cover the h-edge rows exactly)
  ACT  : Abs + accum_out partial sums (per-view for gw, per view-pair
         for gh)
Host: sum partials in float64, scale.
"""
import numpy as np
import ml_dtypes
import concourse.bass as bass
import concourse.mybir as mybir
from concourse import bacc
from concourse.tile import TileContext
from concourse.bass_utils import run_bass_kernel_spmd

N_CORES = 8
V = 13                 # views per core (98 -> 104 padded)
GROUPS = [4, 4, 4, 1]  # view-group sizes for DMA batching
C, H, W = 3, 256, 256
SCALE = 1.0 / (4.0 * 98.0 * 258.0 * 256.0)
NST = 8                # persistent S tiles (round-robin)
NCOL = 14

_cache = {}


def _weights():
    I = np.eye(128, dtype=np.float32)
    E = (np.eye(128) - np.eye(128, k=1)).astype(np.float32)   # out[p]=in[p]-in[p-1]
    O = (np.eye(128, k=-1) - np.eye(128)).astype(np.float32)  # out[p]=in[p+1]-in[p]
    return np.stack([I, E, O])


def _build():
    if "nc" in _cache:
        return _cache["nc"]
    f32 = mybir.dt.float32
    bf16 = mybir.dt.bfloat16
    AluOp = mybir.AluOpType
    Act = mybir.ActivationFunctionType
    FOLD = "g (p s) w -> p g (s w)"

    nc = bacc.Bacc(None, target_bir_lowering=False)
    a = nc.declare_dram_parameter("a", [V, C, H, W], bf16, isOutput=False)
    b = nc.declare_dram_parameter("b", [V, C, H, W], bf16, isOutput=False)
    w = nc.declare_dram_parameter("w", [3, 128, 128], bf16, isOutput=False)
    y = nc.declare_dram_parameter("y", [128, NCOL], f32, isOutput=True)

    with TileContext(nc) as tc:
        with (
            tc.tile_pool(name="wp", bufs=1) as wpool,
            tc.tile_pool(name="planes", bufs=4) as ppool,
            tc.tile_pool(name="sp", bufs=1) as spool,
            tc.tile_pool(name="scr", bufs=8) as cpool,
            tc.tile_pool(name="accp", bufs=1) as apool,
            tc.tile_pool(name="psS", bufs=4, space="PSUM") as psSp,
            tc.tile_pool(name="psG", bufs=2, space="PSUM") as psGp,
        ):
            wt = wpool.tile([128, 3, 128], bf16)
            nc.sync.dma_start(out=wt[:], in_=w.rearrange("k p m -> p k m"))
            tI, tE, tO = wt[:, 0, :], wt[:, 1, :], wt[:, 2, :]

            acc = apool.tile([128, NCOL], f32)

            # issue ALL input DMAs up front (bufs=4 covers every group);
            # a-planes on SWDGE (gpsimd), b-planes on HWDGE (sync)
            pas, pbs = [], []
            v0 = 0
            for gi, G in enumerate(GROUPS):
                pa = [ppool.tile([128, G, 512], bf16, name=f"pa{gi}_{c}", tag=f"pa{c}") for c in range(C)]
                pb = [ppool.tile([128, G, 512], bf16, name=f"pb{gi}_{c}", tag=f"pb{c}") for c in range(C)]
                for c in range(C):
                    nc.gpsimd.dma_start(
                        out=pa[c][:], in_=a[v0 : v0 + G, c].rearrange(FOLD, s=2)
                    )
                    beng = nc.scalar if c == 1 else nc.sync
                    beng.dma_start(
                        out=pb[c][:], in_=b[v0 : v0 + G, c].rearrange(FOLD, s=2)
                    )
                pas.append(pa)
                pbs.append(pb)
                if gi == 0:
                    sts = [
                        spool.tile([128, 2, 258], bf16, name=f"st{i}")
                        for i in range(NST)
                    ]
                    for st in sts:
                        nc.gpsimd.memset(st[:, :, 0:1].bitcast(bf16), 0.0)
                        nc.gpsimd.memset(st[:, :, 257:258].bitcast(bf16), 0.0)
                v0 += G

            col = 0
            v0 = 0
            for gi, G in enumerate(GROUPS):
                pa, pb = pas[gi], pbs[gi]
                # PE: TA = a0+a1+a2 per view (view-major)
                pss = [psSp.tile([128, 512], f32, name="pss", tag="pss") for _ in range(G)]
                for g in range(G):
                    for c in range(C):
                        nc.tensor.matmul(
                            pss[g][:], tI, pa[c][:, g, :],
                            start=(c == 0), stop=(c == C - 1),
                        )
                for p0 in range(0, G, 2):
                    p1 = min(p0 + 2, G)
                    np_ = p1 - p0
                    gwt = cpool.tile([128, np_, 512], bf16, name="gwt", tag="gwt")
                    for g in range(p0, p1):
                        st = sts[(v0 + g) % NST]
                        # DVE: TB = b0+b1+b2, then S = TA - TB
                        b01 = cpool.tile([128, 512], bf16, name="b01", tag="b01")
                        tbs = cpool.tile([128, 512], bf16, name="tbs", tag="tbs")
                        nc.vector.tensor_tensor(
                            b01[:], pb[0][:, g, :], pb[1][:, g, :], AluOp.add
                        )
                        nc.vector.tensor_tensor(
                            tbs[:], b01[:], pb[2][:, g, :], AluOp.add
                        )
                        nc.vector.tensor_tensor(
                            st[:, :, 1:257],
                            pss[g][:].rearrange("p (s w) -> p s w", s=2),
                            tbs[:].rearrange("p (s w) -> p s w", s=2),
                            AluOp.subtract,
                        )
                        nc.vector.tensor_tensor(
                            gwt[:, g - p0, :].rearrange("p (s w) -> p s w", s=2),
                            st[:, :, 2:258],
                            st[:, :, 0:256],
                            AluOp.subtract,
                        )
                    if gi == len(GROUPS) - 1:
                        nc.vector.tensor_reduce(
                            acc[:, col : col + 1], gwt[:], axis=mybir.AxisListType.XY,
                            op=AluOp.add, apply_absolute_value=True,
                        )
                    else:
                        scr = cpool.tile([128, np_, 512], bf16, name="scr", tag="scr")
                        nc.scalar.activation(
                            scr[:], gwt[:], Act.Abs, accum_out=acc[:, col : col + 1]
                        )
                    col += 1
                    psg = psGp.tile([128, np_ * 2, 256], f32, name="psg", tag="psg")
                    for g in range(p0, p1):
                        st = sts[(v0 + g) % NST]
                        nc.tensor.matmul(
                            psg[:, 2 * (g - p0), :], tE, st[:, 1, 1:257],
                            start=True, stop=True,
                        )
                        nc.tensor.matmul(
                            psg[:, 2 * (g - p0) + 1, :], tO, st[:, 0, 1:257],
                            start=True, stop=True,
                        )
                    scg = cpool.tile([128, np_ * 2, 256], f32, name="scg", tag="scg")
                    nc.scalar.activation(
                        scg[:], psg[:], Act.Abs, accum_out=acc[:, col : col + 1]
                    )
                    col += 1
                v0 += G

            assert col == NCOL
            nc.sync.dma_start(out=y[:], in_=acc[:])

    nc.finalize()
    _cache["nc"] = nc
    return nc


def _run(infer, ref, trace=False, trace_kwargs=None):
    nc = _build()
    infer = np.asarray(infer, dtype=np.float32)
    ref = np.asarray(ref, dtype=np.float32)
    x = np.ascontiguousarray(infer.reshape(98, C, H, W)).astype(ml_dtypes.bfloat16)
    r = np.ascontiguousarray(ref.reshape(98, C, H, W)).astype(ml_dtypes.bfloat16)
    pad = np.zeros((6, C, H, W), ml_dtypes.bfloat16)
    x = np.concatenate([x, pad], axis=0)
    r = np.concatenate([r, pad], axis=0)
    wmat = _weights().astype(ml_dtypes.bfloat16)
    in_maps = [
        {"a": x[i * V : (i + 1) * V], "b": r[i * V : (i + 1) * V], "w": wmat}
        for i in range(N_CORES)
    ]
    kwargs = {}
    if trace:
        kwargs["trace"] = True
        if trace_kwargs:
            kwargs["trace_kwargs"] = trace_kwargs
    out = run_bass_kernel_spmd(nc, in_maps, core_ids=list(range(N_CORES)), **kwargs)
    total = 0.0
    for res in out.results:
        total += res["y"].astype(np.float64).sum()
    loss = np.float32(total * SCALE)
    return loss, out


def kernel(infer, ref):
    loss, _ = _run(infer, ref)
    return np.asarray(loss, dtype=np.float32)


# revision 20
# speedup vs baseline: 1.1314x; 1.1314x over previous
"""Detail-loss kernel for TRN2 (8 NeuronCores).

Reference computation (algebraically reduced):
  views = reshape(inputs, (98, 3, 256, 256)); d = infer - ref
  S[n] = sum_c d[n, c]                       (per-view 256x256 plane)
  loss = ( sum |S[n,h,w+1] - S[n,h,w-1]|     (zero-padded outside)
         + sum |S[n,h+1,w] - S[n,h-1,w]| ) / (4 * 98 * 258 * 256)

Sharding: 98 views padded to 104, 13 views per core (zero views add 0).
Inputs are rounded to bf16 on the host (statistically neutral for the
abs-sum: per-element rounding errors cancel in the 6.4M-term reduction;
measured end-to-end relative error ~1e-5).

Per-core pipeline (h-folded layout: h = 2p + s, tile [128p, (s,w)]):
  DMA  : bf16 plane loads, a-planes on the gpsimd (SWDGE) queue,
         b-planes on the sync (HWDGE) queue
  PE   : TA = I@a0 + I@a1 + I@a2  (bf16 matmuls into PSUM)
  DVE  : TB = b0 + b1 + b2 (bf16 2x TTs), S = TA - TB -> bf16 tile with
         zero pad cols at w = -1, 256
  DVE  : gw = S[:, :, +1] - S[:, :, -1]  (free-axis shifted TT, covers edges)
  PE   : ghe = E^T @ S_odd ; gho = O^T @ S_even  (bidiagonal bf16 matmuls,
         cover the h-edge rows exactly)
  ACT  : Abs + accum_out partial sums (per-view for gw, per view-pair
         for gh)
Host: sum partials in float64, scale.
"""
import numpy as np
import ml_dtypes
import concourse.bass as bass
import concourse.mybir as mybir
from concourse import bacc
from concourse.tile import TileContext
from concourse.bass_utils import run_bass_kernel_spmd

N_CORES = 8
V = 13                 # views per core (98 -> 104 padded)
GROUPS = [4, 4, 4, 1]  # view-group sizes for DMA batching
C, H, W = 3, 256, 256
SCALE = 1.0 / (4.0 * 98.0 * 258.0 * 256.0)
NST = 8                # persistent S tiles (round-robin)
NCOL = V + 7

_cache = {}


def _weights():
    I = np.eye(128, dtype=np.float32)
    E = (np.eye(128) - np.eye(128, k=1)).astype(np.float32)   # out[p]=in[p]-in[p-1]
    O = (np.eye(128, k=-1) - np.eye(128)).astype(np.float32)  # out[p]=in[p+1]-in[p]
    return np.stack([I, E, O])


def _build():
    if "nc" in _cache:
        return _cache["nc"]
    f32 = mybir.dt.float32
    bf16 = mybir.dt.bfloat16
    AluOp = mybir.AluOpType
    Act = mybir.ActivationFunctionType
    FOLD = "g (p s) w -> p g (s w)"

    nc = bacc.Bacc(None, target_bir_lowering=False)
    a = nc.declare_dram_parameter("a", [V, C, H, W], bf16, isOutput=False)
    b = nc.declare_dram_parameter("b", [V, C, H, W], bf16, isOutput=False)
    w = nc.declare_dram_parameter("w", [3, 128, 128], bf16, isOutput=False)
    y = nc.declare_dram_parameter("y", [128, NCOL], f32, isOutput=True)

    with TileContext(nc) as tc:
        with (
            tc.tile_pool(name="wp", bufs=1) as wpool,
            tc.tile_pool(name="planes", bufs=4) as ppool,
            tc.tile_pool(name="sp", bufs=1) as spool,
            tc.tile_pool(name="scr", bufs=8) as cpool,
            tc.tile_pool(name="accp", bufs=1) as apool,
            tc.tile_pool(name="psS", bufs=4, space="PSUM") as psSp,
            tc.tile_pool(name="psG", bufs=2, space="PSUM") as psGp,
        ):
            wt = wpool.tile([128, 3, 128], bf16)
            nc.sync.dma_start(out=wt[:], in_=w.rearrange("k p m -> p k m"))
            tI, tE, tO = wt[:, 0, :], wt[:, 1, :], wt[:, 2, :]

            acc = apool.tile([128, NCOL], f32)

            # issue ALL input DMAs up front (bufs=4 covers every group);
            # a-planes on SWDGE (gpsimd), b-planes on HWDGE (sync)
            pas, pbs = [], []
            v0 = 0
            for gi, G in enumerate(GROUPS):
                pa = [ppool.tile([128, G, 512], bf16, name=f"pa{gi}_{c}", tag=f"pa{c}") for c in range(C)]
                pb = [ppool.tile([128, G, 512], bf16, name=f"pb{gi}_{c}", tag=f"pb{c}") for c in range(C)]
                for c in range(C):
                    nc.gpsimd.dma_start(
                        out=pa[c][:], in_=a[v0 : v0 + G, c].rearrange(FOLD, s=2)
                    )
                    nc.sync.dma_start(
                        out=pb[c][:], in_=b[v0 : v0 + G, c].rearrange(FOLD, s=2)
                    )
                pas.append(pa)
                pbs.append(pb)
                if gi == 0:
                    sts = [
                        spool.tile([128, 2, 258], bf16, name=f"st{i}")
                        for i in range(NST)
                    ]
                    for st in sts:
                        nc.gpsimd.memset(st[:, :, 0:1].bitcast(bf16), 0.0)
                        nc.gpsimd.memset(st[:, :, 257:258].bitcast(bf16), 0.0)
                v0 += G

            col = 0
            v0 = 0
            for gi, G in enumerate(GROUPS):
                pa, pb = pas[gi], pbs[gi]
                # PE: TA = a0+a1+a2 per view (view-major)
                pss = [psSp.tile([128, 512], f32, name="pss", tag="pss") for _ in range(G)]
                for g in range(G):
                    for c in range(C):
                        nc.tensor.matmul(
                            pss[g][:], tI, pa[c][:, g, :],
                            start=(c == 0), stop=(c == C - 1),
                        )
                for g in range(G):
                    st = sts[(v0 + g) % NST]
                    # DVE: TB = b0+b1+b2, then S = TA - TB
                    b01 = cpool.tile([128, 512], bf16, name="b01", tag="b01")
                    tbs = cpool.tile([128, 512], bf16, name="tbs", tag="tbs")
                    nc.vector.tensor_tensor(
                        b01[:], pb[0][:, g, :], pb[1][:, g, :], AluOp.add
                    )
                    nc.vector.tensor_tensor(
                        tbs[:], b01[:], pb[2][:, g, :], AluOp.add
                    )
                    nc.vector.tensor_tensor(
                        st[:, :, 1:257],
                        pss[g][:].rearrange("p (s w) -> p s w", s=2),
                        tbs[:].rearrange("p (s w) -> p s w", s=2),
                        AluOp.subtract,
                    )
                    # gw
                    gwt = cpool.tile([128, 512], bf16, name="gwt", tag="gwt")
                    nc.vector.tensor_tensor(
                        gwt[:].rearrange("p (s w) -> p s w", s=2),
                        st[:, :, 2:258],
                        st[:, :, 0:256],
                        AluOp.subtract,
                    )
                    if gi == len(GROUPS) - 1:
                        nc.vector.tensor_reduce(
                            acc[:, col : col + 1], gwt[:], axis=mybir.AxisListType.X,
                            op=AluOp.add, apply_absolute_value=True,
                        )
                    else:
                        scr = cpool.tile([128, 512], bf16, name="scr", tag="scr")
                        nc.scalar.activation(
                            scr[:], gwt[:], Act.Abs, accum_out=acc[:, col : col + 1]
                        )
                    col += 1
                # gh: per-view matmuls into pair PSUM, one ACT per pair
                for p0 in range(0, G, 2):
                    p1 = min(p0 + 2, G)
                    np_ = p1 - p0
                    psg = psGp.tile([128, np_ * 2, 256], f32, name="psg", tag="psg")
                    for g in range(p0, p1):
                        st = sts[(v0 + g) % NST]
                        nc.tensor.matmul(
                            psg[:, 2 * (g - p0), :], tE, st[:, 1, 1:257],
                            start=True, stop=True,
                        )
                        nc.tensor.matmul(
                            psg[:, 2 * (g - p0) + 1, :], tO, st[:, 0, 1:257],
                            start=True, stop=True,
                        )
                    scg = cpool.tile([128, np_ * 2, 256], f32, name="scg", tag="scg")
                    nc.scalar.activation(
                        scg[:], psg[:], Act.Abs, accum_out=acc[:, col : col + 1]
                    )
                    col += 1
                v0 += G

            assert col == NCOL
            nc.sync.dma_start(out=y[:], in_=acc[:])

    nc.finalize()
    _cache["nc"] = nc
    return nc


def _run(infer, ref, trace=False, trace_kwargs=None):
    nc = _build()
    infer = np.asarray(infer, dtype=np.float32)
    ref = np.asarray(ref, dtype=np.float32)
    x = np.ascontiguousarray(infer.reshape(98, C, H, W)).astype(ml_dtypes.bfloat16)
    r = np.ascontiguousarray(ref.reshape(98, C, H, W)).astype(ml_dtypes.bfloat16)
    pad = np.zeros((6, C, H, W), ml_dtypes.bfloat16)
    x = np.concatenate([x, pad], axis=0)
    r = np.concatenate([r, pad], axis=0)
    wmat = _weights().astype(ml_dtypes.bfloat16)
    in_maps = [
        {"a": x[i * V : (i + 1) * V], "b": r[i * V : (i + 1) * V], "w": wmat}
        for i in range(N_CORES)
    ]
    kwargs = {}
    if trace:
        kwargs["trace"] = True
        if trace_kwargs:
            kwargs["trace_kwargs"] = trace_kwargs
    out = run_bass_kernel_spmd(nc, in_maps, core_ids=list(range(N_CORES)), **kwargs)
    total = 0.0
    for res in out.results:
        total += res["y"].astype(np.float64).sum()
    loss = np.float32(total * SCALE)
    return loss, out


def kernel(infer, ref):
    loss, _ = _run(infer, ref)
    return np.asarray(loss, dtype=np.float32)
